# revision 24
# baseline (speedup 1.0000x reference)
"""BoxAttention (3D window attention) Trainium2 Bass kernel, 8-core data-parallel.

Layout strategy (per core, windows sharded 256/core, processed in pairs = 128 tokens):
  - host pre-transposes x -> feature-major xT chunks; q/k computed feature-major
    [4h x 32d, tokens], v token-major; scores computed transposed S_T[m, n] so the
    softmax sum is a PE ones-matmul and AV needs no on-chip transposes.
  - All matmuls keep the contraction at partitions 0..127 (row position fixed at 0;
    varying tile_position rows crashes this runtime).  Per-head score matmuls use
    full-K k-chunks against block-diagonal zero-padded q tiles (built with small
    same-partition SBUF DMAs into memset-once ring buffers); AV uses zero-blocked
    attn tiles, so the cross-window/cross-head rows contribute zero.
  - bias (rel-pos table gather) and per-window mask accumulate into the score PSUM
    via identity-weight matmuls (mask pre-transposed on host, head-replicated by a
    step-0 DRAM read).
  - softmax without max-subtraction (scores bounded, exp safe in fp32 PSUM);
    division deferred: reciprocal of PE-computed sums, row-broadcast via gpsimd,
    one vector multiply.
  - bf16 operands on-chip (PE is 1 cycle/row at any moving size), fp32 PSUM, fp32r
    for the mask add. b_proj added on host.
"""
import os
import numpy as np
import ml_dtypes

BOX = 4
N = BOX ** 3          # 64
DIM = 384
H = 12
HD = 32
SCALE = HD ** -0.5
NCORES = 8

bf16 = ml_dtypes.bfloat16

_cache = {}


def _relative_position_index():
    coords = np.stack(np.meshgrid(*([np.arange(BOX)] * 3), indexing='ij'))
    coords_flat = coords.reshape(3, -1)
    rel = coords_flat[:, :, None] - coords_flat[:, None, :]
    rel = rel.transpose(1, 2, 0) + (BOX - 1)
    rel[..., 0] *= (2 * BOX - 1) * (2 * BOX - 1)
    rel[..., 1] *= (2 * BOX - 1)
    return rel.sum(-1)  # [N, N] int


def _build(wpc):
    """Build the per-core SPMD program for `wpc` windows per core."""
    import concourse.bacc as bacc
    import concourse.tile as tile
    from concourse import mybir
    import concourse.bass as bass
    from contextlib import ExitStack

    T = wpc * N                    # tokens per core
    n_pairs = wpc // 2
    FP32 = mybir.dt.float32
    FP32R = mybir.dt.float32r
    BF16 = mybir.dt.bfloat16
    EXP = mybir.ActivationFunctionType.Exp

    nc = bacc.Bacc("TRN2", target_bir_lowering=False, debug=False)

    xT_d = nc.dram_tensor("xT", [3, 128, T], BF16, kind="ExternalInput")
    maskT_d = nc.dram_tensor("maskT", [wpc, N, N], FP32R, kind="ExternalInput")
    wqkvT_d = nc.dram_tensor("wqkvT", [3, 128, 3 * DIM], BF16, kind="ExternalInput")
    wpT_d = nc.dram_tensor("wpT", [3, 128, DIM], BF16, kind="ExternalInput")
    bias_d = nc.dram_tensor("bias_sb", [3, 128, 256], BF16, kind="ExternalInput")
    i128b_d = nc.dram_tensor("i128b", [128, 128], BF16, kind="ExternalInput")
    i128f_d = nc.dram_tensor("i128f", [128, 128], FP32R, kind="ExternalInput")
    ones2_d = nc.dram_tensor("ones2", [128, 64], BF16, kind="ExternalInput")
    y_d = nc.dram_tensor("y", [T, DIM], FP32, kind="ExternalOutput")

    with tile.TileContext(nc) as tc:
        with ExitStack() as stk:
            singles = stk.enter_context(tc.tile_pool(name="singles", bufs=1))
            xt_pool = stk.enter_context(tc.tile_pool(name="xt", bufs=2))
            qk_ps = stk.enter_context(tc.tile_pool(name="qk_ps", bufs=2, space="PSUM"))
            qk_sb_pool = stk.enter_context(tc.tile_pool(name="qk_sb", bufs=12))
            v_ps = stk.enter_context(tc.tile_pool(name="v_ps", bufs=1, space="PSUM"))
            v_sb_pool = stk.enter_context(tc.tile_pool(name="v_sb", bufs=2))
            mask_pool = stk.enter_context(tc.tile_pool(name="mask", bufs=2))
            s_ps = stk.enter_context(tc.tile_pool(name="s_ps", bufs=2, space="PSUM"))
            smp_ps = stk.enter_context(tc.tile_pool(name="smp_ps", bufs=1, space="PSUM"))
            attn_u_pool = stk.enter_context(tc.tile_pool(name="attn_u", bufs=2))
            r_pool = stk.enter_context(tc.tile_pool(name="r_sb", bufs=2))
            r4_pool = stk.enter_context(tc.tile_pool(name="r4", bufs=2))
            rbt_pool = stk.enter_context(tc.tile_pool(name="rbt", bufs=2))
            rb_pool = stk.enter_context(tc.tile_pool(name="rb", bufs=2))
            av_ps = stk.enter_context(tc.tile_pool(name="av_ps", bufs=1, space="PSUM"))
            av_sb_pool = stk.enter_context(tc.tile_pool(name="av_sb", bufs=2))
            y_ps = stk.enter_context(tc.tile_pool(name="y_ps", bufs=1, space="PSUM"))
            y_sb_pool = stk.enter_context(tc.tile_pool(name="y_sb", bufs=2))

            # ---- constants ----
            wq_sb = []
            for g in range(3):
                t_ = singles.tile([128, 3 * DIM], BF16, tag=f"wq{g}")
                nc.sync.dma_start(out=t_, in_=wqkvT_d[g])
                wq_sb.append(t_)
            wp_sb = []
            for g in range(3):
                t_ = singles.tile([128, DIM], BF16, tag=f"wp{g}")
                nc.sync.dma_start(out=t_, in_=wpT_d[g])
                wp_sb.append(t_)
            bias_sb = []
            for g in range(3):
                t_ = singles.tile([128, 256], BF16, tag=f"bias{g}")
                nc.sync.dma_start(out=t_, in_=bias_d[g])
                bias_sb.append(t_)
            i128b = singles.tile([128, 128], BF16, tag="i128b")
            nc.sync.dma_start(out=i128b, in_=i128b_d[:, :])
            i128f = singles.tile([128, 128], FP32R, tag="i128f")
            nc.sync.dma_start(out=i128f, in_=i128f_d[:, :])
            ones2 = singles.tile([128, 64], BF16, tag="ones2")
            nc.sync.dma_start(out=ones2, in_=ones2_d[:, :])

            # zero-padded ring buffers (content blocks rewritten per pair,
            # zero blocks written exactly once here)
            qz_ring = []      # [ring][g] -> [128=(4sl,32d), 512=(4sl)(2w)(64n)]
            attn2_ring = []   # [ring] -> [128=(2w,64m), 1536=(2w)(12h)(64n)]
            for rix in range(2):
                qzg = []
                for g in range(3):
                    t_ = singles.tile([128, 512], BF16, tag=f"qz{rix}_{g}")
                    nc.vector.memset(t_, 0.0)
                    qzg.append(t_)
                qz_ring.append(qzg)
                t_ = singles.tile([128, 1536], BF16, tag=f"attn2_{rix}")
                nc.vector.memset(t_, 0.0)
                attn2_ring.append(t_)

            for s in range(n_pairs // 2):      # super-block = 2 pairs = 256 tokens
                xt = xt_pool.tile([128, 3, 256], BF16)
                for g in range(3):
                    nc.sync.dma_start(
                        out=xt[:, g, :], in_=xT_d[g, :, 256 * s:256 * s + 256]
                    )
                # q/k chunks feature-major: cg 0..2 = q (heads 4cg..), 3..5 = k
                qk = []
                for cg in range(6):
                    ps = qk_ps.tile([128, 256], FP32)
                    for g in range(3):
                        nc.tensor.matmul(
                            ps,
                            lhsT=wq_sb[g][:, 128 * cg:128 * cg + 128],
                            rhs=xt[:, g, :],
                            start=(g == 0),
                            stop=(g == 2),
                        )
                    sb = qk_sb_pool.tile([128, 256], BF16, tag="qk")
                    nc.vector.tensor_copy(sb, ps)
                    qk.append(sb)

                for pl in range(2):            # pair within super-block
                    p = 2 * s + pl
                    pc = 128 * pl              # column offset in SB tiles
                    qz = qz_ring[p % 2]
                    attn2 = attn2_ring[p % 2]

                    # ---- v token-major [128=(2w,64m), (12h)(32d)] ----
                    vps = v_ps.tile([128, 384], FP32)
                    for g in range(3):
                        nc.tensor.matmul(
                            vps,
                            lhsT=xt[:, g, pc:pc + 128],
                            rhs=wq_sb[g][:, 768:1152],
                            start=(g == 0),
                            stop=(g == 2),
                        )
                    v_sb = v_sb_pool.tile([128, 384], BF16)
                    nc.scalar.copy(v_sb, vps)

                    # ---- block-diag q tiles: diag block sl of qz[g] holds
                    # qT(head 4g+sl) [32, (2w)(64n)]; off-diag stays zero ----
                    for g in range(3):
                        for sl in range(4):
                            nc.sync.dma_start(
                                out=qz[g][32 * sl:32 * sl + 32,
                                          128 * sl:128 * sl + 128],
                                in_=qk[g][32 * sl:32 * sl + 32, pc:pc + 128],
                            )

                    # ---- per-pair mask, replicated over the 4 head-slots by
                    # a step-0 DRAM read: [128=(2w,64m), (4sl)(64n)] ----
                    mk = mask_pool.tile([128, 256], FP32R)
                    base = maskT_d[2 * p:2 * p + 2]
                    src = bass.AP(
                        tensor=base.tensor,
                        offset=base.offset,
                        ap=[[N * N, 2], [N, N], [0, 4], [1, N]],
                    )
                    nc.sync.dma_start(out=mk, in_=src)

                    # ---- scores S_T per chunk g: [128=(2w,64m), (4sl)(64n)] ----
                    attn_u = attn_u_pool.tile([128, 768], BF16)
                    for g in range(3):
                        sp = s_ps.tile([128, 256], FP32, tag="sp")
                        nc.tensor.matmul(
                            sp, lhsT=i128b, rhs=bias_sb[g], start=True, stop=False,
                            skip_group_check=True,
                        )
                        nc.tensor.matmul(
                            sp, lhsT=i128f, rhs=mk[:, :], start=False, stop=False,
                            skip_group_check=True,
                        )
                        qz_r = qz[g][:, :].rearrange("p (a b) -> p a b", a=4)
                        for w in range(2):
                            rhs = qz_r[:, :, 64 * w:64 * w + 64]
                            nc.tensor.matmul(
                                sp[64 * w:64 * w + 64, :],
                                lhsT=qk[3 + g][:, pc + 64 * w:pc + 64 * w + 64],
                                rhs=rhs,
                                start=False,
                                stop=True,
                                tile_position=(0, 64 * w),
                                skip_group_check=True,
                            )
                        # ---- exp ----
                        nc.scalar.activation(
                            out=attn_u[:, 256 * g:256 * g + 256],
                            in_=sp,
                            func=EXP,
                        )

                    # ---- softmax sums via ones-matmul; reciprocal ----
                    smp = smp_ps.tile([128, 384], FP32)
                    for th in range(2):
                        for w in range(2):
                            k = 2 * th + w
                            nc.tensor.matmul(
                                smp[32 * k:32 * k + 32, :],
                                lhsT=ones2[:, 32 * w:32 * w + 32],
                                rhs=attn_u[:, 384 * th:384 * th + 384],
                                start=True,
                                stop=True,
                                tile_position=(0, 32 * k),
                            )
                    r_sb = r_pool.tile([128, 384], BF16)
                    with nc.allow_low_precision(reason="softmax recip bf16 ok"):
                        nc.vector.reciprocal(r_sb, smp[:, :])

                    # ---- broadcast r rows across the 64 m-partitions.
                    # HW partition_broadcast only honors partition 0 on both
                    # ends: stage r rows to partition 0, broadcast into
                    # partitions 0-63, and DMA the w=1 half up to 64-127. ----
                    r4 = r4_pool.tile([1, 4, 384], BF16)
                    for k in range(4):
                        nc.sync.dma_start(
                            out=r4[0:1, k, :],
                            in_=r_sb[32 * k:32 * k + 1, :],
                        )
                    rb = rb_pool.tile([128, 768], BF16)
                    rbt = rbt_pool.tile([64, 768], BF16)
                    for th in range(2):
                        # w = 0 goes straight into rb rows 0-63
                        nc.gpsimd.partition_broadcast(
                            rb[0:64, 384 * th:384 * th + 384],
                            r4[0:1, 2 * th, :],
                        )
                        # w = 1 lands in rbt rows 0-63, then moves up
                        nc.gpsimd.partition_broadcast(
                            rbt[0:64, 384 * th:384 * th + 384],
                            r4[0:1, 2 * th + 1, :],
                        )
                    nc.sync.dma_start(out=rb[64:128, :], in_=rbt[:, :])

                    # ---- scale + write into zero-blocked attn2 ----
                    for w in range(2):
                        nc.vector.tensor_mul(
                            attn2[64 * w:64 * w + 64, 768 * w:768 * w + 768],
                            attn_u[64 * w:64 * w + 64, :],
                            rb[64 * w:64 * w + 64, :],
                        )

                    # ---- AV: out[d, n] at av[(4sl)(32d), (3g)(2w)(64n)] ----
                    avp = av_ps.tile([128, 384], FP32)
                    for w in range(2):
                        for h in range(H):
                            g, sl = h // 4, h % 4
                            nc.tensor.matmul(
                                avp[32 * sl:32 * sl + 32,
                                    128 * g + 64 * w:128 * g + 64 * w + 64],
                                lhsT=v_sb[:, 32 * h:32 * h + 32],
                                rhs=attn2[:, 768 * w + 64 * h:768 * w + 64 * h + 64],
                                start=True,
                                stop=True,
                                tile_position=(0, 32 * sl),
                            )
                    av_sb = av_sb_pool.tile([128, 384], BF16)
                    nc.scalar.copy(av_sb, avp)

                    # ---- proj (token-major y) ----
                    yp = y_ps.tile([128, 384], FP32)
                    for g in range(3):
                        nc.tensor.matmul(
                            yp,
                            lhsT=av_sb[:, 128 * g:128 * g + 128],
                            rhs=wp_sb[g],
                            start=(g == 0),
                            stop=(g == 2),
                        )
                    y_sb = y_sb_pool.tile([128, 384], FP32)
                    nc.vector.tensor_copy(y_sb, yp)
                    nc.sync.dma_start(
                        out=y_d[128 * p:128 * p + 128, :], in_=y_sb
                    )

    nc.finalize()
    return nc


def _host_prep(x, mask, w_qkv, bias_table):
    """Build per-core in_maps. x [B,64,384] f32, mask [nW,64,64] f32."""
    B = x.shape[0]
    wpc = B // NCORES

    wqkvT = w_qkv.T.astype(np.float32).copy()          # [384, 1152]
    wqkvT[:, :DIM] *= SCALE
    wqkvT_b = np.ascontiguousarray(
        wqkvT.reshape(3, 128, 3 * DIM).astype(bf16))

    rel = _relative_position_index()
    bias_full = bias_table[rel.reshape(-1)].reshape(N, N, H)  # [n, m, H]
    bias_sb = np.zeros((3, 128, 256), np.float32)
    for g in range(3):
        for sl in range(4):
            h = 4 * g + sl
            bT = bias_full[:, :, h].T                  # [m, n]
            for w in range(2):
                bias_sb[g, 64 * w:64 * w + 64, 64 * sl:64 * sl + 64] = bT
    bias_sb = bias_sb.astype(bf16)

    i128b = np.eye(128, dtype=np.float32).astype(bf16)
    i128f = np.eye(128, dtype=np.float32)
    ones2 = np.zeros((128, 64), np.float32)
    ones2[:64, 0] = 1.0     # w=0 indicator in col 0 of block 0
    ones2[64:, 32] = 1.0    # w=1 indicator in col 0 of block 1
    ones2 = ones2.astype(bf16)

    in_maps = []
    for c in range(NCORES):
        xs = x[c * wpc:(c + 1) * wpc].reshape(wpc * N, DIM)
        xT = np.ascontiguousarray(xs.T.astype(bf16).reshape(3, 128, wpc * N))
        ms = mask[(c % 2) * wpc:(c % 2) * wpc + wpc]
        maskT = np.ascontiguousarray(
            ms.transpose(0, 2, 1).astype(np.float32))
        in_maps.append({
            "xT": xT,
            "maskT": maskT,
            "wqkvT": wqkvT_b,
            "wpT": None,          # filled by kernel()
            "bias_sb": bias_sb,
            "i128b": i128b,
            "i128f": i128f,
            "ones2": ones2,
        })
    return in_maps, wpc


def kernel(x, mask, w_qkv, bias_table, w_proj, b_proj):
    from concourse.bass_utils import run_bass_kernel_spmd

    x = np.asarray(x, dtype=np.float32)
    mask = np.asarray(mask, dtype=np.float32)
    w_qkv = np.asarray(w_qkv, dtype=np.float32)
    bias_table = np.asarray(bias_table, dtype=np.float32)
    w_proj = np.asarray(w_proj, dtype=np.float32)
    b_proj = np.asarray(b_proj, dtype=np.float32)

    B = x.shape[0]
    in_maps, wpc = _host_prep(x, mask, w_qkv, bias_table)
    wpT_b = np.ascontiguousarray(
        w_proj.T.reshape(3, 128, DIM).astype(bf16))
    for m in in_maps:
        m["wpT"] = wpT_b

    key = (B,)
    if key not in _cache:
        _cache[key] = _build(wpc)
    nc = _cache[key]

    trace = bool(int(os.environ.get("BASS_KERNEL_TRACE", "0")))
    res = run_bass_kernel_spmd(nc, in_maps, core_ids=list(range(NCORES)),
                               trace=trace)
    if trace:
        kernel.last_exec_time_ns = res.exec_time_ns
        kernel.last_results = res

    out = np.concatenate(
        [res.results[c]["y"].reshape(wpc, N, DIM) for c in range(NCORES)],
        axis=0,
    ).astype(np.float32)
    out += b_proj[None, None, :]
    return out
